# revision 15
# baseline (speedup 1.0000x reference)
"""MoE FFN (8 experts, top-2) on 8 Trainium2 NeuronCores.

Hidden-dimension sharding for perfect load balance: every core processes ALL
16384 (token, expert) pairs, but only H/8 = 512 of each expert's 4096 hidden
units. Per-core work is exactly 2048 token-equivalents regardless of routing
skew (per-expert counts vary +-130, which under expert parallelism pads every
core to the max count). Each core holds H-slice weights of all 8 experts
(16.8 MB bf16, SBUF-resident) and emits a partial output in bf16; the host
sums the 8 partials and applies the combine weights. b2 is fed as b2/8 so the
partial sum reproduces the bias exactly once.

The token stream is sorted by expert; tile boundaries are specialized to the
routing at build time (compile cache keyed on the per-expert counts), so tiles
never straddle an expert boundary and no capacity padding exists anywhere.

Each HWDGE queue drains serially in program order, so a y-output descriptor
(gated on the tile's evictions) parked at the queue head starves later x
prefetches — that head-of-line blocking cost 145us of PE idle and dropped HAM
to half clock. Inputs therefore ride the Activation-engine HWDGE queue (x two
tiles ahead; expert e's weight chunks spread just-in-time across the previous
expert's tiles) while outputs ride the SP queue (one fused DMA per tile).

On-device layout: all matmul operands keep the contraction dim on SBUF
partitions. PSUM accumulates in f32; layer-1 bias rides the gelu on ScalarE,
layer-2 bias (pre-divided by 8) is fused into the PSUM eviction on VectorE,
which also downcasts the partial to bf16.
"""

import numpy as np
import ml_dtypes

N_EXPERTS = 8
TOP_K = 2
C = 1024
H = 4096
HS = H // N_EXPERTS      # per-core hidden slice
P = 128
T_TILE = 512
KO1 = C // P             # 8 contraction chunks for layer 1
KO2 = HS // P            # 4 contraction chunks for layer 2
MO1 = HS // P            # 4 output chunks for layer 1
CO2 = C // P             # 8 output chunks for layer 2
TP = 16384               # total (token, expert) pairs: 8192 tokens * top-2

_nc_cache = {}


def _tile_plan(counts):
    """Balanced single-expert tiles over the expert-sorted pair stream.

    The first expert leads with a small tile: compute then starts on ~0.3 MB
    of DMA instead of ~2 MB, which warms the HAM throttle and the DMA ramp
    while the bulk of the first x/w1 stream is still in flight."""
    tiles = []
    t0 = 0
    first = True
    for e, c in enumerate(counts):
        if c == 0:
            continue
        lead = min(128, c) if first else 0
        first = False
        if lead:
            tiles.append((e, t0, lead))
            t0 += lead
            c -= lead
        k = -(-c // T_TILE)
        for i in range(k):
            T = c // k + (1 if i < c % k else 0)
            if T == 0:
                continue
            tiles.append((e, t0, T))
            t0 += T
    # small final tile: the post-stream drain (evictions + output DMA of the
    # last tile) scales with its width
    if tiles and tiles[-1][2] > 256:
        e, t0_, T = tiles.pop()
        tiles.append((e, t0_, T - 128))
        tiles.append((e, t0_ + T - 128, 128))
    assert sum(t for _, _, t in tiles) == sum(counts)
    return tiles


def _build_nc(counts):
    import concourse.mybir as mybir
    import concourse.tile as tile
    from concourse import bacc

    bf16 = mybir.dt.bfloat16
    f32 = mybir.dt.float32

    nc = bacc.Bacc()
    xt = nc.dram_tensor("xt", [C, TP], bf16, kind="ExternalInput")
    w1 = [
        nc.dram_tensor(f"w1_{e}", [C, HS], bf16, kind="ExternalInput")
        for e in range(N_EXPERTS)
    ]
    w2 = [
        nc.dram_tensor(f"w2_{e}", [HS, C], bf16, kind="ExternalInput")
        for e in range(N_EXPERTS)
    ]
    # biases host-pre-swizzled and packed: [P, e*MO1+m] / [P, e*CO2+co]
    b1 = nc.dram_tensor("b1all", [P, N_EXPERTS * MO1], f32, kind="ExternalInput")
    b2 = nc.dram_tensor("b2all", [P, N_EXPERTS * CO2], f32, kind="ExternalInput")
    yt = nc.dram_tensor("yt", [C, TP], bf16, kind="ExternalOutput")

    xt_r = xt.rearrange("(ko ki) t -> ki ko t", ki=P)
    w1_r = [w.rearrange("(ko ki) h -> ki ko h", ki=P) for w in w1]
    w2_r = [w.rearrange("(ko ki) c -> ki ko c", ki=P) for w in w2]
    yt_r = yt.rearrange("(co p) t -> p co t", p=P)

    tiles = _tile_plan(counts)
    n_tiles = len(tiles)
    used = []
    for e, _, _ in tiles:
        if e not in used:
            used.append(e)
    gelu = mybir.ActivationFunctionType.Gelu_apprx_tanh

    with tile.TileContext(nc) as tc:
        with (
            tc.tile_pool(name="const", bufs=1) as const,
            tc.tile_pool(name="xp", bufs=4) as xp,
            tc.tile_pool(name="gp", bufs=2) as gp,
            tc.tile_pool(name="yp", bufs=2) as yp,
            tc.tile_pool(name="psum", bufs=8, space="PSUM") as psum,
        ):
            w1_sb = {
                e: const.tile([P, KO1, HS], bf16, tag=f"w1_{e}", name=f"w1s{e}")
                for e in used
            }
            w2_sb = {
                e: const.tile([P, KO2, C], bf16, tag=f"w2_{e}", name=f"w2s{e}")
                for e in used
            }
            b1_sb = const.tile([P, N_EXPERTS * MO1], f32, tag="b1")
            b2_sb = const.tile([P, N_EXPERTS * CO2], f32, tag="b2")

            # HAM warmup: the PE cold-starts clock-gated at half rate and only
            # reaches 8/8 after ~4us of sustained activity. Burn that ramp on
            # dummy matmuls over a memset scratch tile while the first x/w
            # DMAs are still in flight, so real tiles run at full clock.
            warm = const.tile([P, P], bf16, tag="warm")
            nc.gpsimd.memset(warm[:], 0)
            pw = psum.tile([P, T_TILE], mybir.dt.float32, tag="ps", name="pwarm")
            for _ in range(36):
                nc.tensor.matmul(pw[:, :P], warm[:], warm[:], start=True, stop=True)

            # Just-in-time weight schedule: expert used[ui]'s two fused DMAs
            # are emitted spread across the tiles of expert used[ui-1], so the
            # serial HWDGE never buries an upcoming x prefetch behind bulk
            # weight traffic (that cost 65us of PE idle in v1). Experts 0-1
            # load in the prologue instead, split across both HWDGE queues.
            tiles_of = {}
            for ti, (e, _, _) in enumerate(tiles):
                tiles_of.setdefault(e, []).append(ti)
            sched = {ti: [] for ti in range(n_tiles)}
            for ui in range(1, len(used)):
                e = used[ui]
                slots = tiles_of[used[ui - 1]]
                if ui == 1 and len(slots) > 2:
                    # keep the first expert's early tiles weight-free: t<38us
                    # is DMA-ramp-starved and any extra traffic there turns
                    # into PE idle and a HAM half-clock penalty
                    slots = slots[2:]
                chunks = [
                    (w1_sb[e][:, :, :], w1_r[e][:, :, :]),
                    (w2_sb[e][:, :, :], w2_r[e][:, :, :]),
                ]
                for ci, ch in enumerate(chunks):
                    sched[slots[min(ci * len(slots) // len(chunks), len(slots) - 1)]].append(ch)

            # Prologue. SP queue: tile-0 x interleaved with first expert's w1
            # (per-ko chunks so the first layer-1 chain starts on 0.3MB, in
            # halves so m=0/1 need not wait for the full slab). Activation
            # queue, racing in parallel: biases, first expert's w2, next two
            # x tiles, second expert's weights.
            e0, t00, T0 = tiles[0]
            x_tiles = {}
            x_tiles[0] = xp.tile([P, KO1, T_TILE], bf16, tag="x", name="x0")
            for ko in range(KO1):
                nc.sync.dma_start(
                    x_tiles[0][:, ko : ko + 1, :T0],
                    xt_r[:, ko : ko + 1, t00 : t00 + T0],
                )
                nc.sync.dma_start(
                    w1_sb[e0][:, ko : ko + 1, 0 : HS // 2],
                    w1_r[e0][:, ko : ko + 1, 0 : HS // 2],
                )
            for ko in range(KO1):
                nc.sync.dma_start(
                    w1_sb[e0][:, ko : ko + 1, HS // 2 :],
                    w1_r[e0][:, ko : ko + 1, HS // 2 :],
                )
            nc.scalar.dma_start(b1_sb[:], b1[:])
            nc.scalar.dma_start(b2_sb[:], b2[:])
            nc.scalar.dma_start(w2_sb[e0][:, :, :], w2_r[e0][:, :, :])
            for tj in (1, 2):
                if tj < n_tiles:
                    ne, nt0, nt = tiles[tj]
                    x_tiles[tj] = xp.tile(
                        [P, KO1, T_TILE], bf16, tag="x", name=f"x{tj}"
                    )
                    nc.scalar.dma_start(
                        x_tiles[tj][:, :, :nt], xt_r[:, :, nt0 : nt0 + nt]
                    )

            for ti, (e, t0, T) in enumerate(tiles):
                # prefetch x three tiles ahead, then this slot's weight chunks,
                # all on the Activation HWDGE queue (outputs own the SP queue)
                if ti + 3 < n_tiles:
                    ne, nt0, nt = tiles[ti + 3]
                    x_tiles[ti + 3] = xp.tile(
                        [P, KO1, T_TILE], bf16, tag="x", name=f"x{ti + 3}"
                    )
                    nc.scalar.dma_start(
                        x_tiles[ti + 3][:, :, :nt], xt_r[:, :, nt0 : nt0 + nt]
                    )
                for dst, src in sched[ti]:
                    nc.scalar.dma_start(dst, src)

                x_sb = x_tiles.pop(ti)
                g_sb = gp.tile([P, KO2, T_TILE], bf16, tag="g")
                for m in range(MO1):
                    ph = psum.tile([P, T_TILE], mybir.dt.float32, tag="ps")
                    for ko in range(KO1):
                        nc.tensor.matmul(
                            ph[:, :T],
                            w1_sb[e][:, ko, m * P : (m + 1) * P],
                            x_sb[:, ko, :T],
                            start=(ko == 0),
                            stop=(ko == KO1 - 1),
                        )
                    nc.scalar.activation(
                        g_sb[:, m, :T],
                        ph[:, :T],
                        gelu,
                        bias=b1_sb[:, e * MO1 + m : e * MO1 + m + 1],
                    )
                y_sb = yp.tile([P, CO2, T_TILE], bf16, tag="y")
                for co in range(CO2):
                    py = psum.tile([P, T_TILE], mybir.dt.float32, tag="ps")
                    for ho in range(KO2):
                        nc.tensor.matmul(
                            py[:, :T],
                            w2_sb[e][:, ho, co * P : (co + 1) * P],
                            g_sb[:, ho, :T],
                            start=(ho == 0),
                            stop=(ho == KO2 - 1),
                        )
                    nc.vector.tensor_scalar_add(
                        y_sb[:, co, :T],
                        py[:, :T],
                        b2_sb[:, e * CO2 + co : e * CO2 + co + 1],
                    )
                    # per-co output DMA drains progressively during compute;
                    # the small final tile instead fuses all 8 chunks into one
                    # descriptor — 8 serial ~0.7us issue slots after the last
                    # matmul were pure tail
                    if ti + 1 < n_tiles:
                        nc.sync.dma_start(
                            yt_r[:, co, t0 : t0 + T], y_sb[:, co, :T]
                        )
                if ti + 1 == n_tiles:
                    nc.sync.dma_start(
                        yt_r[:, :, t0 : t0 + T], y_sb[:, :, :T]
                    )
    nc.finalize()
    return nc


def _route(flat_f32: np.ndarray, gate_w: np.ndarray):
    """Router, bit-matching the reference's jax ops (same env/backend)."""
    import jax
    import jax.numpy as jnp

    logits = jnp.asarray(flat_f32) @ jnp.asarray(gate_w).T
    probs = jax.nn.softmax(logits, axis=-1)
    top_p, top_i = jax.lax.top_k(probs, TOP_K)
    weights = top_p / (jnp.sum(top_p, axis=-1, keepdims=True) + 1e-8)
    return np.asarray(top_i), np.asarray(weights)


# results of the last device run, for test harness introspection
last_result = None


def _ensure_ntff_hook():
    """bass_utils' trace path imports antenv.axon_hooks, which the agent
    image's antenv lacks. Build the hook from trn_agent_boot's ctypes
    shim and inject a stand-in module."""
    import sys
    import types

    if "antenv.axon_hooks" in sys.modules:
        return
    try:
        from trn_agent_boot.trn_boot import _ntff_profile_via_ctypes

        hook = _ntff_profile_via_ctypes("/opt/axon/libaxon_pjrt.so")
    except Exception:
        hook = None
    m = types.ModuleType("antenv.axon_hooks")
    m.get_axon_ntff_profile_hook = lambda: hook
    m.set_axon_ntff_profile_hook = lambda h: None
    sys.modules["antenv.axon_hooks"] = m


def kernel(x, gate_w, w1, b1, w2, b2):
    from concourse.bass_utils import run_bass_kernel_spmd

    x = np.asarray(x)
    B, N, _ = x.shape
    flat = np.ascontiguousarray(x.reshape(-1, C), dtype=np.float32)
    T = flat.shape[0]
    assert T * TOP_K == TP

    top_i, weights = _route(flat, np.asarray(gate_w, dtype=np.float32))

    # expert-sorted pair stream
    tok_e = []
    wgt_e = []
    for e in range(N_EXPERTS):
        rows, cols = np.nonzero(top_i == e)
        tok_e.append(rows.astype(np.int64))
        wgt_e.append(weights[rows, cols].astype(np.float32))
    counts = tuple(len(i) for i in tok_e)
    pair_tok = np.concatenate(tok_e)
    pair_w = np.concatenate(wgt_e)

    nc = _nc_cache.get(counts)
    if nc is None:
        nc = _build_nc(counts)
        _nc_cache[counts] = nc

    bf16 = ml_dtypes.bfloat16
    xs = np.ascontiguousarray(flat[pair_tok].T).astype(bf16)  # [C, TP]
    w1 = np.asarray(w1, dtype=np.float32)
    w2 = np.asarray(w2, dtype=np.float32)
    b1 = np.asarray(b1, dtype=np.float32)
    b2 = np.asarray(b2, dtype=np.float32)

    in_maps = []
    for k in range(N_EXPERTS):
        hs = slice(k * HS, (k + 1) * HS)
        b1k = np.ascontiguousarray(
            b1[:, hs].reshape(N_EXPERTS, MO1, P).transpose(2, 0, 1).reshape(P, -1)
        )
        b2k = np.ascontiguousarray(
            (b2 / N_EXPERTS).reshape(N_EXPERTS, CO2, P).transpose(2, 0, 1).reshape(P, -1)
        )
        im = {"xt": xs, "b1all": b1k, "b2all": b2k}
        for e in range(N_EXPERTS):
            im[f"w1_{e}"] = np.ascontiguousarray(w1[e, hs, :].T).astype(bf16)
            im[f"w2_{e}"] = np.ascontiguousarray(w2[e, :, hs].T).astype(bf16)
        in_maps.append(im)

    import os

    trace = bool(int(os.environ.get("MOE_TRACE", "0")))
    if trace:
        _ensure_ntff_hook()

    global last_result
    res = run_bass_kernel_spmd(
        nc,
        in_maps,
        core_ids=list(range(N_EXPERTS)),
        trace=trace,
    )
    last_result = res

    ysum = np.zeros((C, TP), dtype=np.float32)
    for k in range(N_EXPERTS):
        ysum += res.results[k]["yt"].astype(np.float32)
    contrib = (ysum * pair_w[None, :]).T  # [TP, C]
    out = np.zeros((T, C), dtype=np.float32)
    np.add.at(out, pair_tok, contrib)
    return out.reshape(B, N, C)
